# revision 24
# baseline (speedup 1.0000x reference)
"""Trainium2 Bass kernel for the CECL contrastive loss (nn_CeclLossModule).

Strategy (8 NeuronCores, SPMD):
  - N = B*A = 6400 rows, D = 256. Core c owns global rows [800c, 800c+800)
    (padded to 896 = 7*128; pad rows wrap around and are discarded on host).
  - Each core receives the full embedding/time arrays ROTATED by 800c so
    that (a) its own rows are the first 896 columns, letting the lhsT tiles
    be slices of the shared normalized-transposed matrix FT, and (b) the
    "same-sample" 8-wide block-diagonal of its row-block always sits at
    local columns [128t, 128t+128) for row-tile t, identical on all cores
    (required because all cores run one identical program).
  - Per row-tile of 128 rows: z = F_rows @ F.T via fp32r matmuls into PSUM.
    Valid-negative mask nov[i,j] = [max(sf_i,sf_j) > min(ef_i,ef_j)] is
    built from two tensor_scalar ops (GPSIMD) + one fused
    tensor_tensor_reduce (DVE) that also accumulates the per-row count.
    zm = z + BIG*nov, and one ScalarE Softplus with fused scale/bias
    (bias' = bias - scale*BIG) and fused row-sum accumulation yields
    sum_j nov * softplus(scale*z+bias) directly: masked (overlapping)
    elements see softplus(-large) = 0.
  - The positive-pair terms and the in-group corrections are handled on the
    static 128x128 diagonal block per row-tile with three small fused
    reduces against constant EQ / (EQ - I) patterns.
  - Per-row nll = rowsum / count computed on-device; host gathers the 8x896
    vectors, drops padding, and takes the mean.
"""

import numpy as np

N = 6400
D = 256
A = 8
NCORES = 8
RPC = 800          # rows per core
RT = 7             # row tiles per core (896 rows, 96 pad)
RTP = RT * 128     # 896
CTW = 512          # col tile width
NCT = 13           # 12*512 + 256
BIG = 2048.0

_cached = {}


def build():
    """Build the full Bass program. Returns nc."""
    import concourse.bass as bass
    import concourse.bacc as bacc
    import concourse.tile as tile
    from concourse import mybir
    from contextlib import ExitStack

    f32 = mybir.dt.float32
    f32r = mybir.dt.float32r
    ALU = mybir.AluOpType
    ACTF = mybir.ActivationFunctionType
    AX = mybir.AxisListType

    nc = bacc.Bacc("TRN2", target_bir_lowering=False)
    ecols = nc.declare_dram_parameter("ecols", [N, D], f32, isOutput=False)
    sfc = nc.declare_dram_parameter("sfc", [N], f32, isOutput=False)
    efc = nc.declare_dram_parameter("efc", [N], f32, isOutput=False)
    eqcd = nc.declare_dram_parameter("eqc", [128, 128], f32, isOutput=False)
    poscd = nc.declare_dram_parameter("posc", [128, 128], f32, isOutput=False)
    idnd = nc.declare_dram_parameter("idn", [128, 128], f32, isOutput=False)
    scld = nc.declare_dram_parameter("scl", [1], f32, isOutput=False)
    biad = nc.declare_dram_parameter("bia", [1], f32, isOutput=False)
    nlld = nc.declare_dram_parameter("nll", [RTP], f32, isOutput=True)

    with ExitStack() as ctx:
        tc = ctx.enter_context(tile.TileContext(nc))

        singles = ctx.enter_context(tc.tile_pool(name="singles", bufs=1))
        epool = ctx.enter_context(tc.tile_pool(name="e", bufs=3))
        fpool = ctx.enter_context(tc.tile_pool(name="f", bufs=3))
        sspool = ctx.enter_context(tc.tile_pool(name="ss", bufs=3))
        tp_psum = ctx.enter_context(tc.tile_pool(name="tpp", bufs=2, space="PSUM"))
        z_psum = ctx.enter_context(tc.tile_pool(name="zp", bufs=4, space="PSUM"))
        mpool = ctx.enter_context(tc.tile_pool(name="m", bufs=3))
        t12pool = ctx.enter_context(tc.tile_pool(name="t12", bufs=2))
        zmpool = ctx.enter_context(tc.tile_pool(name="zm", bufs=2))
        sppool = ctx.enter_context(tc.tile_pool(name="sp", bufs=2))
        smallpool = ctx.enter_context(tc.tile_pool(name="small", bufs=4))
        partpool = ctx.enter_context(tc.tile_pool(name="part", bufs=2))

        # ----- constants / scalars -----
        eqc_t = singles.tile([128, 128], f32)
        nc.sync.dma_start(out=eqc_t, in_=eqcd[:, :])
        posc_t = singles.tile([128, 128], f32)
        nc.sync.dma_start(out=posc_t, in_=poscd[:, :])
        idn_t = singles.tile([128, 128], f32)
        nc.sync.dma_start(out=idn_t, in_=idnd[:, :])

        scl_t = singles.tile([128, 1], f32)
        nc.gpsimd.dma_start(out=scl_t, in_=scld[:].to_broadcast([128, 1]))
        bia_t = singles.tile([128, 1], f32)
        nc.gpsimd.dma_start(out=bia_t, in_=biad[:].to_broadcast([128, 1]))
        # bias_eff = bias - BIG*scale ; nscl = -scale ; nbia = -bias
        bias_eff = singles.tile([128, 1], f32)
        nc.vector.scalar_tensor_tensor(
            out=bias_eff, in0=scl_t, scalar=-BIG, in1=bia_t,
            op0=ALU.mult, op1=ALU.add)
        nscl_t = singles.tile([128, 1], f32)
        nc.vector.tensor_scalar_mul(nscl_t, scl_t, -1.0)
        nbia_t = singles.tile([128, 1], f32)
        nc.vector.tensor_scalar_mul(nbia_t, bia_t, -1.0)

        # per-row start/end times: sfp[p, t] = sf[128t + p]
        sfp = singles.tile([128, 50], f32)
        nc.sync.dma_start(out=sfp, in_=sfc.rearrange("(t p) -> p t", p=128))
        efp = singles.tile([128, 50], f32)
        nc.sync.dma_start(out=efp, in_=efc.rearrange("(t p) -> p t", p=128))

        # ----- phase 1a: broadcast sf/ef to all 128 partitions (DMA) -----
        SFB = singles.tile([128, N], f32)
        EFB = singles.tile([128, N], f32)
        import concourse.bass as bass_mod
        for ct in range(NCT):
            off = ct * CTW
            w = min(CTW, N - off)
            for src, dst in ((sfc, SFB), (efc, EFB)):
                sl = src[off:off + w]
                bcast = bass_mod.AP(tensor=sl.tensor, offset=sl.offset,
                                    ap=[[0, 128]] + list(sl.ap))
                nc.gpsimd.dma_start(out=dst[:, off:off + w], in_=bcast)

        # ----- phase 1b: normalize embeddings + transpose into FT -----
        # FT flat layout: chunk k of row-tile t lives at cols 256*t + 128*k.
        FT = singles.tile([128, 2 * N], f32r)
        for m in range(25):
            tp = tp_psum.tile([128, 512], f32, tag="tp")
            for j in range(2):
                t = 2 * m + j
                et = epool.tile([128, D], f32, tag="et")
                nc.sync.dma_start(out=et, in_=ecols[128 * t:128 * t + 128, :])
                sq = fpool.tile([128, D], f32, tag="sq")
                ss = sspool.tile([128, 1], f32, tag="ss")
                nc.vector.scalar_tensor_tensor(
                    out=sq, in0=et, scalar=1.0, in1=et,
                    op0=ALU.mult, op1=ALU.mult, accum_out=ss)
                lnss = sspool.tile([128, 1], f32, tag="lnss")
                nc.scalar.activation(lnss, ss, ACTF.Ln)
                inv = sspool.tile([128, 1], f32, tag="inv")
                nc.scalar.activation(inv, lnss, ACTF.Exp, scale=-0.5)
                fn = fpool.tile([128, D], f32, tag="fn")
                nc.vector.tensor_scalar_mul(fn, et, inv)
                for k in range(2):
                    nc.tensor.transpose(
                        tp[:, 256 * j + 128 * k:256 * j + 128 * k + 128],
                        fn[:, 128 * k:128 * k + 128], idn_t)
            nc.vector.tensor_copy(FT[:, 512 * m:512 * m + 512], tp)

        # ----- phase 2: bulk row-block loss -----
        FTv = FT.rearrange("p (t k c) -> p t k c", k=2, c=128)
        ones_f32 = singles.tile([128, 2], f32)
        nc.vector.memset(ones_f32, 1.0)
        ones_col = singles.tile([128, 2], f32r)
        nc.vector.tensor_copy(ones_col, ones_f32)
        zr_psum = ctx.enter_context(
            tc.tile_pool(name="zrp", bufs=2, space="PSUM"))
        nllb = singles.tile([128, RT], f32)
        for rt in range(RT):
            sf_i = sfp[:, rt:rt + 1]
            ef_i = efp[:, rt:rt + 1]
            s1parts = partpool.tile([128, NCT], f32, tag="s1p")
            zmparts = partpool.tile([128, NCT], f32, tag="zmp")
            negc = smallpool.tile([128, 1], f32, tag="negc")
            cntc = smallpool.tile([128, 1], f32, tag="cntc")
            posc_acc = smallpool.tile([128, 1], f32, tag="posa")
            # zrow = sum_j z_mm[i, j] via ones matvec (for count recovery).
            # fp32r matmuls need an even moving dim, so use 2 ones columns.
            zrow2 = zr_psum.tile([128, 2], f32, tag="zrow")
            for k in range(2):
                nc.tensor.matmul(
                    zrow2,
                    lhsT=FT[:, 256 * rt + 128 * k:256 * rt + 128 * k + 128],
                    rhs=ones_col, start=(k == 0), stop=(k == 1))
            zrow = zrow2[:, 0:1]

            zts = []
            # matmuls in groups of 4 col-tiles to keep weights stationary
            for ctg in range(0, NCT, 4):
                cts = range(ctg, min(ctg + 4, NCT))
                ztiles = {}
                for ct in cts:
                    ztiles[ct] = z_psum.tile([128, CTW], f32, tag="z",
                                             name=f"z{rt}_{ct}")
                for k in range(2):
                    lhsT = FT[:, 256 * rt + 128 * k:256 * rt + 128 * k + 128]
                    for ct in cts:
                        off = ct * CTW
                        w = min(CTW, N - off)
                        nt = w // 128
                        rhs = FTv[:, 4 * ct:4 * ct + nt, k, :]
                        nc.tensor.matmul(
                            ztiles[ct][:, :w], lhsT=lhsT, rhs=rhs,
                            start=(k == 0), stop=(k == 1))
                for ct in cts:
                    zts.append((ct, ztiles[ct]))

            ct_diag = (128 * rt) // CTW
            sub = 128 * rt - ct_diag * CTW
            for ct, zt in zts:
                off = ct * CTW
                w = min(CTW, N - off)
                p = mpool.tile([128, CTW], f32, tag="p")
                nc.gpsimd.tensor_scalar(
                    out=p[:, :w], in0=SFB[:, off:off + w],
                    scalar1=sf_i, scalar2=None, op0=ALU.max)
                q = mpool.tile([128, CTW], f32, tag="q")
                nc.gpsimd.tensor_scalar(
                    out=q[:, :w], in0=EFB[:, off:off + w],
                    scalar1=ef_i, scalar2=None, op0=ALU.min)
                t12 = t12pool.tile([128, CTW], f32, tag="t12")
                nc.vector.tensor_tensor(
                    out=t12[:, :w], in0=p[:, :w], in1=q[:, :w], op=ALU.is_gt)
                zm = zmpool.tile([128, CTW], f32, tag="zmt")
                nc.vector.scalar_tensor_tensor(
                    out=zm[:, :w], in0=t12[:, :w], scalar=BIG, in1=zt[:, :w],
                    op0=ALU.mult, op1=ALU.add,
                    accum_out=zmparts[:, ct:ct + 1])
                ex = sppool.tile([128, CTW], f32, tag="ex")
                nc.scalar.activation(
                    ex[:, :w], zm[:, :w], ACTF.Exp,
                    bias=bias_eff, scale=scl_t)
                spm = sppool.tile([128, CTW], f32, tag="spm")
                nc.scalar.activation(
                    spm[:, :w], ex[:, :w], ACTF.Ln, bias=1.0,
                    accum_out=s1parts[:, ct:ct + 1])

                if ct == ct_diag:
                    # corrections on the in-group 128x128 diagonal block
                    scr = mpool.tile([128, 128], f32, tag="scr")
                    nc.vector.scalar_tensor_tensor(
                        out=scr, in0=eqc_t, scalar=1.0,
                        in1=spm[:, sub:sub + 128],
                        op0=ALU.mult, op1=ALU.mult, accum_out=negc)
                    scr2 = mpool.tile([128, 128], f32, tag="scr2")
                    nc.vector.scalar_tensor_tensor(
                        out=scr2, in0=eqc_t, scalar=1.0,
                        in1=t12[:, sub:sub + 128],
                        op0=ALU.mult, op1=ALU.mult, accum_out=cntc)
                    exn = smallpool.tile([128, 128], f32, tag="exn")
                    nc.scalar.activation(
                        exn, zt[:, sub:sub + 128], ACTF.Exp,
                        bias=nbia_t, scale=nscl_t)
                    spn = smallpool.tile([128, 128], f32, tag="spn")
                    nc.scalar.activation(spn, exn, ACTF.Ln, bias=1.0)
                    scr3 = mpool.tile([128, 128], f32, tag="scr3")
                    nc.vector.scalar_tensor_tensor(
                        out=scr3, in0=posc_t, scalar=1.0, in1=spn,
                        op0=ALU.mult, op1=ALU.mult, accum_out=posc_acc)

            # ----- per-row finalization -----
            s1 = smallpool.tile([128, 1], f32, tag="s1")
            nc.vector.reduce_sum(s1, s1parts, axis=AX.X)
            zmsum = smallpool.tile([128, 1], f32, tag="zmsum")
            nc.vector.reduce_sum(zmsum, zmparts, axis=AX.X)
            # rowsum = s1 - negc + posc_acc
            rowsum = smallpool.tile([128, 1], f32, tag="rows")
            nc.vector.scalar_tensor_tensor(
                out=rowsum, in0=s1, scalar=negc, in1=posc_acc,
                op0=ALU.subtract, op1=ALU.add)
            # sum(t12)*BIG = zmsum - zrow ; cnt = sum(t12) - cntc + 7
            t12sum = smallpool.tile([128, 1], f32, tag="t12sum")
            nc.vector.tensor_tensor(
                out=t12sum, in0=zmsum, in1=zrow, op=ALU.subtract)
            cnt = smallpool.tile([128, 1], f32, tag="cnt")
            nc.vector.scalar_tensor_tensor(
                out=cnt, in0=t12sum, scalar=1.0 / BIG, in1=cntc,
                op0=ALU.mult, op1=ALU.subtract)
            cnt7 = smallpool.tile([128, 1], f32, tag="cnt7")
            nc.vector.tensor_scalar_add(cnt7, cnt, float(A - 1))
            icnt = smallpool.tile([128, 1], f32, tag="icnt")
            nc.vector.reciprocal(icnt, cnt7)
            nc.vector.tensor_tensor(
                out=nllb[:, rt:rt + 1], in0=rowsum, in1=icnt, op=ALU.mult)

        for rt in range(RT):
            nc.sync.dma_start(out=nlld[128 * rt:128 * rt + 128],
                              in_=nllb[:, rt:rt + 1])
    nc.compile()
    return nc


def _get_nc():
    if "nc" not in _cached:
        _cached["nc"] = build()
    return _cached["nc"]


def kernel(embeddings, start_times, end_times, logit_scale, logit_bias):
    from concourse.bass_utils import run_bass_kernel_spmd

    emb = np.ascontiguousarray(np.asarray(embeddings), dtype=np.float32).reshape(N, D)
    sf = np.ascontiguousarray(np.asarray(start_times), dtype=np.float32).reshape(N)
    ef = np.ascontiguousarray(np.asarray(end_times), dtype=np.float32).reshape(N)
    scl = np.asarray(logit_scale, dtype=np.float32).reshape(1)
    bia = np.asarray(logit_bias, dtype=np.float32).reshape(1)

    gid = np.arange(128) // A
    eqc = (gid[:, None] == gid[None, :]).astype(np.float32)
    posc = eqc - np.eye(128, dtype=np.float32)
    idn = np.eye(128, dtype=np.float32)

    in_maps = []
    for c in range(NCORES):
        rot = np.roll(np.arange(N), -RPC * c)
        in_maps.append({
            "ecols": np.ascontiguousarray(emb[rot]),
            "sfc": np.ascontiguousarray(sf[rot]),
            "efc": np.ascontiguousarray(ef[rot]),
            "eqc": eqc, "posc": posc, "idn": idn,
            "scl": scl, "bia": bia,
        })

    nc = _get_nc()
    res = run_bass_kernel_spmd(nc, in_maps, list(range(NCORES)), **_run_opts)
    _cached["last_result"] = res
    nll = np.concatenate([res.results[c]["nll"][:RPC] for c in range(NCORES)])
    return np.float32(nll.mean())


# test-harness knob: test.py sets _run_opts["trace"] = True to get exec_time_ns
_run_opts = {}
